# revision 98
# baseline (speedup 1.0000x reference)
"""Swin-style window attention kernel for 8 TRN2 NeuronCores.

Sharding: data-parallel over batch B=32 -> 4 images per core. No collectives.

Per-core dataflow (B_local=4 images, 384ch x 56x56, WS=7, 12 heads, d=32):
  stripe = (image b, window-row wr): 7x56 = 392 pixels = 8 windows.

PSUM rule (hardware): a (bank, partition-range) may only be written by ONE
PE row-group (tile_position row band). Layout obeying it in 8 banks:
  - mm pool: 2 banks (qkv chunks / proj / borrowed by V^T transposes)
  - sps: 2 x 2-bank tiles [128, 2(hqi), 4, 128] -- hqi stride = full bank,
    so each bank is written by a single head row-group. One exp op per tile.
  - obm: 2 x 1-bank tiles keyed by WINDOW PARITY (parity == row-group):
    AV out partitions = channel (co=32*hq), wp slots 0:4; denominators
    (ones-stationary) packed at cols 256:452 of the same bank.

Dataflow per stripe:
  1. x loaded via gpsimd casting DMAs (f32->fp8e4 for q/k conv, f32->bf16
     for v conv). 2. q/k conv fp8e4 DoubleRow (K=256 + K=128 tail, weights
  host-scaled x64, descale folded into exp scale); v conv bf16; copies
  permute raster -> window-major padded 64 (pads zeroed for k,v).
  3. V^T via PE transposes into bf16 PSUM borrowed from mm pool; one ACT
  copy per hg. 4. QK^T lhsT=K rhs=Q -> S^T, window-parity bands on
  partitions. 5. exp on ACT (scale folds fp8 descale + d^-0.5), es *=
  exp(bias) on DVE. 6. denom + AV per (hg, parity). 7. DVE reciprocal +
  fused normalize/unband -> attn bf16. 8. proj bf16 + bias via ACT
  unpermute; batched store on SP.

Emission is software-pipelined: iteration i emits phase1(i-1) interleaved
with chunks(i) per hg (covers exp/bias latency with independent PE work),
then phase2(i-1), then proj(i-2).
"""

import os
import numpy as np
import ml_dtypes

import concourse.bass as bass
import concourse.tile as tile
from concourse import bacc, mybir
from concourse.bass_utils import run_bass_kernel_spmd
from concourse.masks import make_identity

F32 = mybir.dt.float32
BF16 = mybir.dt.bfloat16
FP8 = mybir.dt.float8e4

B_LOC = 4      # images per core
C = 384        # channels
H = W = 56
WS = 7         # window size
NH = 12        # heads
D = 32         # head dim
NW = 8         # windows per stripe (56/7)
NPIX = WS * W  # 392 pixels per stripe
WW = WS * WS   # 49
WP = 64        # padded window stride

USE_FP8 = True
FP8_SCALE = 64.0
SEXP = (D ** -0.5) / (FP8_SCALE * FP8_SCALE) if USE_FP8 else D ** -0.5

_CACHE = {}
LAST_EXEC_NS = None


def _rel_index(ws):
    coords = np.stack(np.meshgrid(np.arange(ws), np.arange(ws), indexing='ij')).reshape(2, -1)
    rel = (coords[:, :, None] - coords[:, None, :]).transpose(1, 2, 0).astype(np.int64)
    rel[..., 0] += ws - 1
    rel[..., 1] += ws - 1
    rel[..., 0] *= 2 * ws - 1
    return rel.sum(-1)


def build_bass():
    nc = bacc.Bacc("TRN2", target_bir_lowering=False, debug=False, num_devices=8)

    x_d = nc.dram_tensor("x", [B_LOC, C, H, W], F32, kind="ExternalInput")
    if USE_FP8:
        w8dr_d = nc.dram_tensor("w8dr", [128, 2, 2 * C], FP8, kind="ExternalInput")
        w8dr2_d = nc.dram_tensor("w8dr2", [128, 2, 2 * C], FP8, kind="ExternalInput")
    else:
        wqkT_d = nc.dram_tensor("wqkT", [C, 2 * C], BF16, kind="ExternalInput")
    wvT_d = nc.dram_tensor("wvT", [C, C], BF16, kind="ExternalInput")
    wprojT_d = nc.dram_tensor("wprojT", [C, C], BF16, kind="ExternalInput")
    expb_d = nc.dram_tensor("expb", [128, 3, 2, 2, WW], BF16, kind="ExternalInput")
    bproj_d = nc.dram_tensor("bproj", [C], F32, kind="ExternalInput")
    out_d = nc.dram_tensor("out", [B_LOC, C, H, W], F32, kind="ExternalOutput")

    DR = mybir.MatmulPerfMode.DoubleRow

    with tile.TileContext(nc) as tc:
        with (
            tc.tile_pool(name="singles", bufs=1) as singles,
            tc.tile_pool(name="xp", bufs=3) as xp,
            tc.tile_pool(name="qkvp", bufs=3) as qkvp,
            tc.tile_pool(name="vtp", bufs=2) as vtp,
            tc.tile_pool(name="ep", bufs=2) as ep,
            tc.tile_pool(name="rp", bufs=2) as rp,
            tc.tile_pool(name="ap_", bufs=3) as ap_,
            tc.tile_pool(name="yp", bufs=3) as yp,
            tc.tile_pool(name="mm_ps", bufs=2, space="PSUM") as mm_ps,
            tc.tile_pool(name="s_ps", bufs=1, space="PSUM") as s_ps,
            tc.tile_pool(name="o_ps", bufs=1, space="PSUM") as o_ps,
        ):
            # ---- preload constants ----
            if USE_FP8:
                w8dr_sb = singles.tile([128, 2, 2 * C], FP8)
                nc.sync.dma_start(out=w8dr_sb, in_=w8dr_d.ap())
                w8dr2_sb = singles.tile([128, 2, 2 * C], FP8)
                nc.sync.dma_start(out=w8dr2_sb, in_=w8dr2_d.ap())
            else:
                wqkT_sb = singles.tile([128, 3, 2 * C], BF16)
                nc.sync.dma_start(
                    out=wqkT_sb, in_=wqkT_d.ap().rearrange("(kc p) m -> p kc m", p=128))
            wvT_sb = singles.tile([128, 3, C], BF16)
            nc.sync.dma_start(out=wvT_sb, in_=wvT_d.ap().rearrange("(kc p) m -> p kc m", p=128))
            wprojT_sb = singles.tile([128, 3, C], BF16)
            nc.sync.dma_start(out=wprojT_sb, in_=wprojT_d.ap().rearrange("(kc p) m -> p kc m", p=128))
            expb_sb = singles.tile([128, 3, 2, 2, WW], BF16)
            nc.sync.dma_start(out=expb_sb, in_=expb_d.ap())
            bproj_sb = singles.tile([128, 3], F32)
            nc.sync.dma_start(out=bproj_sb, in_=bproj_d.ap().rearrange("(oc p) -> p oc", p=128))
            ones_sb = singles.tile([128, D], BF16)
            nc.vector.memset(ones_sb, 1.0)
            ident_sb = singles.tile([128, 128], BF16)
            make_identity(nc, ident_sb)

            _lim = int(os.environ.get("STRIPE_LIMIT", "0"))
            stripes = [(b, wr) for b in range(B_LOC) for wr in range(8)]
            if _lim:
                stripes = stripes[:_lim]

            def emit_x_load(b, wr):
                src = x_d[b, :, wr * WS:(wr + 1) * WS, :] \
                    .rearrange("(kc p) r w -> p kc (r w)", p=128)
                st = {}
                if USE_FP8:
                    x8 = xp.tile([128, 4, 448], FP8, tag="x8", name="x8")
                    nc.gpsimd.dma_start(out=x8[:, :3, :NPIX], in_=src)
                    # K padded 384->512 so both q/k matmuls run DoubleRow;
                    # weight tile 3 is zero, but 0*NaN would poison PSUM, so
                    # the junk plane must be finite
                    nc.gpsimd.memset(x8[:, 3, :NPIX], 0.0)
                    st["x8"] = x8
                xb = xp.tile([128, 3, NPIX], BF16, tag="xb", name="xb")
                nc.gpsimd.dma_start(out=xb, in_=src)
                st["xb"] = xb
                st["qs"] = qkvp.tile([128, 3, NW, WP], BF16, tag="qs", name="qs")
                st["ks"] = qkvp.tile([128, 3, NW, WP], BF16, tag="ks", name="ks")
                st["vs"] = qkvp.tile([128, 3, NW, WP], BF16, tag="vs", name="vs")
                nc.gpsimd.memset(st["ks"][:, :, :, WW:], 0.0)
                nc.gpsimd.memset(st["vs"][:, :, :, WW:], 0.0)
                return st

            def emit_chunks_hg(st, hg, which_list=(0, 1, 2)):
                """qkv conv chunks for one head-group + PSUM->SBUF copies."""
                targets = {0: st["qs"], 1: st["ks"], 2: st["vs"]}
                for which in which_list:
                    dst = targets[which]
                    ps_full = mm_ps.tile([128, 512], F32, tag="mmps", name="mmps")
                    ps = ps_full[:, :NPIX]
                    if which < 2 and USE_FP8:
                        co = which * C + hg * 128
                        nc.tensor.matmul(
                            ps, lhsT=w8dr_sb[:, :, co:co + 128],
                            rhs=st["x8"][:, 0:2, :NPIX],
                            perf_mode=DR, start=True, stop=False)
                        nc.tensor.matmul(
                            ps, lhsT=w8dr2_sb[:, :, co:co + 128],
                            rhs=st["x8"][:, 2:4, :NPIX],
                            perf_mode=DR, start=False, stop=True)
                    elif which < 2:
                        co = which * C + hg * 128
                        for kc in range(3):
                            nc.tensor.matmul(
                                ps, lhsT=wqkT_sb[:, kc, co:co + 128],
                                rhs=st["xb"][:, kc],
                                start=(kc == 0), stop=(kc == 2))
                    else:
                        for kc in range(3):
                            nc.tensor.matmul(
                                ps, lhsT=wvT_sb[:, kc, hg * 128:(hg + 1) * 128],
                                rhs=st["xb"][:, kc],
                                start=(kc == 0), stop=(kc == 2))
                    # raster (r w c) -> window-major (w r c)
                    psrc = ps.rearrange("p (r w c) -> p w r c", r=WS, w=NW, c=WS)
                    o = dst[:, hg, :, :WW].rearrange("p w (r c) -> p w r c", r=WS)
                    if which == 1:
                        nc.scalar.copy(out=o, in_=psrc)
                    else:
                        nc.vector.tensor_copy(out=o, in_=psrc)

            def _qk_block(st, hg, sps, hqp):
                for hq in (2 * hqp, 2 * hqp + 1):
                    for w in range(NW):
                        po = WP * (w % 2)
                        nc.tensor.matmul(
                            sps[hqp][po:po + WP, hq % 2, w // 2, :WW],
                            lhsT=st["ks"][hq * D:(hq + 1) * D, hg, w, :],
                            rhs=st["qs"][hq * D:(hq + 1) * D, hg, w, :WW],
                            tile_position=(hq * D, po))
                nc.scalar.activation(
                    out=st["es"][hg][:, hqp], in_=sps[hqp][:, :, :, :WW],
                    func=mybir.ActivationFunctionType.Exp, scale=SEXP)

            def emit_p1a(st, hg):
                """V^T transposes + QK (hq 0,1) + exp0."""
                if hg == 0:
                    st["attn"] = ap_.tile([128, 3, NW, WW], BF16, tag="attn", name="attn")
                    st["vt"] = []
                    st["es"] = []
                    st["sps"] = []
                vtps = o_ps.tile([128, 4, 128], BF16, tag=f"obm{hg % 2}", name="vtps")
                for wp in range(4):
                    nc.tensor.transpose(
                        vtps[:, wp],
                        st["vs"][:, hg, 2 * wp:2 * wp + 2, :].rearrange("p a b -> p (a b)"),
                        ident_sb)
                st["vtps"] = vtps
                sps = [s_ps.tile([128, 2, 4, 128], F32, tag=f"sps{i}", name=f"sps{i}")
                       for i in range(2)]
                st["sps"] = sps
                es = ep.tile([128, 2, 2, 4, WW], BF16, tag="es", name="es", bufs=4)
                st["es"].append(es)
                _qk_block(st, hg, sps, 0)

            def emit_p1b(st, hg):
                """QK (hq 2,3) + exp1 + vt copy + bias."""
                _qk_block(st, hg, st["sps"], 1)
                vt = vtp.tile([128, 4, 128], BF16, tag="vt", name="vt", bufs=4)
                nc.scalar.copy(out=vt, in_=st["vtps"])
                es = st["es"][hg]
                nc.vector.tensor_mul(
                    out=es, in0=es,
                    in1=expb_sb[:, hg, :, :, None, :].to_broadcast((128, 2, 2, 4, WW)))
                st["vt"].append(vt)

            def emit_phase2_hg(st, hg):
                """denom + AV + reciprocal + normalize for one head-group."""
                es, vt = st["es"][hg], st["vt"][hg]
                rinv = rp.tile([128, 2, 4, WW], F32, tag="rinv", name="rinv", bufs=3)
                for par in range(2):
                    po = WP * par
                    obm = o_ps.tile([128, NW, WP], F32, tag=f"obm{par}",
                                    name=f"obm{par}")
                    for hq in range(4):
                        co = D * hq
                        nc.tensor.matmul(
                            obm[co:co + D, 4:8, :]
                            .rearrange("p a b -> p (a b)")[:, :4 * WW],
                            lhsT=ones_sb[po:po + WW, :],
                            rhs=es[po:po + WW, hq // 2, hq % 2],
                            tile_position=(po, co))
                    for wp in range(4):
                        w = 2 * wp + par
                        for hq in range(4):
                            co = D * hq
                            nc.tensor.matmul(
                                obm[co:co + D, wp, :WW],
                                lhsT=vt[po:po + WW, wp, hq * D:(hq + 1) * D],
                                rhs=es[po:po + WW, hq // 2, hq % 2, wp],
                                tile_position=(po, co))
                    nc.vector.reciprocal(
                        out=rinv[:, par],
                        in_=obm[:, 4:8, :].rearrange("p a b -> p (a b)")[:, :4 * WW]
                        .rearrange("p (b n) -> p b n", b=4))
                    nc.vector.tensor_mul(
                        out=st["attn"][:, hg]
                        .rearrange("p (wp par) n -> p par wp n", par=2)[:, par],
                        in0=obm[:, 0:4, :WW],
                        in1=rinv[:, par])

            def emit_proj_oc(st, oc):
                if oc == 0:
                    st["y"] = yp.tile([128, 3, NPIX], F32, tag="y", name="y")
                attn_sb, y_sb = st["attn"], st["y"]
                yps_full = mm_ps.tile([128, 512], F32, tag="mmps", name="mmps")
                yps = yps_full[:, :NPIX]
                for kc in range(3):
                    nc.tensor.matmul(
                        yps, lhsT=wprojT_sb[:, kc, oc * 128:(oc + 1) * 128],
                        rhs=attn_sb[:, kc],
                        start=(kc == 0), stop=(kc == 2))
                nc.scalar.activation(
                    out=y_sb[:, oc].rearrange("p (r w c) -> p r w c", r=WS, w=NW),
                    in_=yps.rearrange("p (w r c) -> p r w c", w=NW, r=WS, c=WS),
                    func=mybir.ActivationFunctionType.Identity,
                    bias=bproj_sb[:, oc:oc + 1])
                if oc == 2:
                    b, wr = st["bwr"]
                    nc.sync.dma_start(
                        out=out_d[b, :, wr * WS:(wr + 1) * WS, :]
                        .rearrange("(oc p) r w -> p oc (r w)", p=128),
                        in_=y_sb)

            # 3-stage software pipeline: phase1(i-1) interleaved with
            # chunks(i) per hg, then phase2(i-1), then proj(i-2)
            cur = None    # stripe i-1 state (chunks emitted, attn pending)
            done = None   # stripe i-2 state (attn emitted, proj pending)
            for i in range(len(stripes) + 2):
                nxt = None
                if i < len(stripes):
                    b, wr = stripes[i]
                    nxt = emit_x_load(b, wr)
                    nxt["bwr"] = (b, wr)

                for hg in range(3):
                    if cur is not None:
                        emit_p1a(cur, hg)
                    if nxt is not None:
                        emit_chunks_hg(nxt, hg, (0,))
                    if cur is not None:
                        emit_p1b(cur, hg)
                    if nxt is not None:
                        emit_chunks_hg(nxt, hg, (1, 2))
                for hg in range(3):
                    if cur is not None:
                        emit_phase2_hg(cur, hg)
                    if done is not None:
                        emit_proj_oc(done, hg)
                done, cur = cur, nxt
    nc.compile()
    return nc


def host_prep(w_qkv, bias_table, w_proj, b_proj):
    w_qk = w_qkv[0:2 * C].copy()                # [768, 384]
    if not USE_FP8:
        w_qk[0:C] *= D ** -0.5
    wqkT = np.ascontiguousarray(w_qk.T)         # [384, 768]
    if USE_FP8:
        w8 = (wqkT * FP8_SCALE).astype(ml_dtypes.float8_e4m3)
        w8p = np.zeros((512, 2 * C), ml_dtypes.float8_e4m3)
        w8p[:C] = w8
        wqk = {
            "w8dr": np.ascontiguousarray(
                w8p[:256].reshape(2, 128, 2 * C).transpose(1, 0, 2)),
            "w8dr2": np.ascontiguousarray(
                w8p[256:].reshape(2, 128, 2 * C).transpose(1, 0, 2)),
        }
    else:
        wqk = {"wqkT": wqkT.astype(ml_dtypes.bfloat16)}
    wvT = np.ascontiguousarray(w_qkv[2 * C:].T).astype(ml_dtypes.bfloat16)
    wprojT = np.ascontiguousarray(w_proj.T).astype(ml_dtypes.bfloat16)

    rel = _rel_index(WS)
    bias = bias_table[rel.reshape(-1)].reshape(WW, WW, NH)   # [n, m, h]
    expbT = np.exp(bias.astype(np.float64)).transpose(1, 2, 0)  # [m, h, n]
    # [128(m banded), 3(hg), 2(hqp), 2(hqi), 49(n)], pad rows zero
    expb = np.zeros((128, 3, 2, 2, WW), np.float64)
    for hg in range(3):
        for hq in range(4):
            h = 4 * hg + hq
            expb[0:WW, hg, hq // 2, hq % 2] = expbT[:, h, :]
            expb[64:64 + WW, hg, hq // 2, hq % 2] = expbT[:, h, :]
    return (wqk, wvT, wprojT, expb.astype(ml_dtypes.bfloat16),
            np.ascontiguousarray(b_proj, dtype=np.float32))


def kernel(x, w_qkv, bias_table, w_proj, b_proj):
    global LAST_EXEC_NS
    x = np.ascontiguousarray(x, dtype=np.float32)
    wqk, wvT, wprojT, expb, bproj = host_prep(
        np.asarray(w_qkv, np.float32), np.asarray(bias_table, np.float32),
        np.asarray(w_proj, np.float32), np.asarray(b_proj, np.float32))

    if "nc" not in _CACHE:
        _CACHE["nc"] = build_bass()
    nc = _CACHE["nc"]

    in_maps = []
    for i in range(8):
        in_maps.append({
            "x": x[B_LOC * i:B_LOC * (i + 1)],
            "wvT": wvT, "wprojT": wprojT,
            "expb": expb, "bproj": bproj, **wqk,
        })
    res = run_bass_kernel_spmd(nc, in_maps, core_ids=list(range(8)), trace=False)
    LAST_EXEC_NS = res.exec_time_ns
    out = np.concatenate([res.results[i]["out"] for i in range(8)], axis=0)
    return out
